# revision 3
# baseline (speedup 1.0000x reference)
"""Trainium2 Bass kernel for nn_DCT_Forward_Model (JPEG-style DCT quantize/dequantize).

Math: the reference output equals the approx_dct forward path:
  B = img - 128 (per 8x8 block), t22 = (X @ B @ X^T)/sf^2 with X = f32(D*65000),
  q = round(t22/Q50 + 1e-6), deq = Q50*q, t2 = (X^T @ deq @ X)/sf^2, out = round(t2)+128.
(The grad path g cancels: out = g + stopgrad(a - g) == a up to fp noise.)

v2 design (per NeuronCore, pure data parallel, 2500 images/core):
  - 5 groups x 500 images (4 subtiles of TI=125)
  - PE transposes in fp32r (1.5 cyc/row vs 2.0 for fp32), N padded to 128
  - forward 2D DCT: fused Kronecker matmuls in fp32r (1 cyc/row at N=500,
    vs 4 for fp32) with 1/Q50 quant scale folded into W1 -> t22' = t22/Q50
  - quantize: ONE DVE tensor_scalar (add MAGIC+c, sub MAGIC) -> bf16 q,
    where c = -64 on block-DC partitions folds the "-128" DC shift exactly
  - inverse DCT in bf16 (q integers exact), data stationary, out in natural
    [img, pixel] layout, PSUM half-tiles [125,512] for double buffering
  - output epilogue: ONE fused op (add MAGIC, sub MAGIC-128) -> int16;
    subs 0,1 split ACT(+MAGIC) + Pool(-sub) [Pool can't read PSUM],
    subs 2,3 one DVE two-op each
  - loads on sync HWDGE ring, stores on scalar HWDGE ring
"""

import os
import sys
import numpy as np
from contextlib import ExitStack

if "/opt/trn_rl_repo" not in sys.path and os.path.isdir("/opt/trn_rl_repo"):
    sys.path.insert(0, "/opt/trn_rl_repo")

N_CORES = 8
SIZE = 20000
PER_CORE = SIZE // N_CORES  # 2500
TI = 125                    # images per subtile
GROUP = 4                   # subtiles per group (fwd N = 500)
NT = PER_CORE // TI         # 20 subtiles per core
NG = NT // GROUP            # 5 groups
MAGIC = 12582912.0          # 1.5 * 2^23: fp32 add snaps to integer (RNE)

_Q50 = np.array(
    [[16, 11, 10, 16, 24, 40, 51, 61], [12, 12, 14, 19, 26, 58, 60, 55],
     [14, 13, 16, 24, 40, 57, 69, 56], [14, 17, 22, 29, 51, 87, 80, 62],
     [18, 22, 37, 56, 68, 109, 103, 77], [24, 35, 55, 64, 81, 104, 113, 92],
     [49, 64, 78, 87, 103, 121, 120, 101], [72, 92, 95, 98, 112, 100, 103, 99]],
    dtype=np.float32)


def _dct_mat8():
    k = np.arange(8)[:, None]
    n = np.arange(8)[None, :]
    D = np.cos(np.pi * k * (2 * n + 1) / 16.0)
    D[0] *= np.sqrt(1.0 / 8.0)
    D[1:] *= np.sqrt(2.0 / 8.0)
    return D.astype(np.float32)


def _build_constants(weight=None, wf=65000.0):
    SF = np.float64(wf)
    if weight is None:
        Xbase = _dct_mat8()
    else:
        Xbase = np.asarray(weight, dtype=np.float32).reshape(8, 8)
    X = (Xbase * np.float32(wf)).astype(np.float32)
    X64 = X.astype(np.float64)
    Q64 = _Q50.astype(np.float64)

    ii_, kk = np.arange(4), np.arange(32)
    jj_, cc = np.arange(4), np.arange(32)
    blkmask = (cc[:, None] // 8 == kk[None, :] // 8)  # [c, k]

    # W1[(jj,c), m=(p_*2+qi), (ii,k)] = X[i8,j8]*X[k8,c8] / (sf^2 * Q50[i8,k8])
    # (quant scale folded in; i8 = (4*p_+ii)%8 is the coef row of output
    #  partition (ii,k) of chunk p_, k8 = k%8 the coef col)
    W1 = np.zeros((128, 16, 128), dtype=np.float64)
    for p_ in range(8):
        jb = p_ // 2
        for qi in range(2):
            q = 2 * jb + qi
            m = p_ * 2 + qi
            i8 = (4 * p_ + ii_) % 8
            j8 = (4 * q + jj_) % 8
            a = X64[i8[None, :], j8[:, None]]            # [jj, ii]
            b = np.where(blkmask, X64[kk[None, :] % 8, cc[:, None] % 8], 0.0)  # [c,k]
            # scale per output element (ii, k): 1 / Q50[i8[ii], k%8]
            scale_out = 1.0 / Q64[i8[:, None], (kk % 8)[None, :]]   # [ii, k]
            w = np.einsum('ji,ck->jcik', a, b) / (SF * SF)
            w = w * scale_out[None, None, :, :]
            W1[:, m, :] = w.reshape(128, 128)

    # W2[(jj,c), q, m2=(i-8jb)*32+k] = X[j8,i8]*X[c8,k8]*Q50[j8,c8]/sf^2
    W2 = np.zeros((128, 8, 256), dtype=np.float64)
    i8_ = np.arange(8)
    for q in range(8):
        j8 = (4 * q + jj_) % 8
        a = X64[j8[:, None], i8_[None, :] % 8]           # [jj, i8]
        b = np.where(blkmask, X64[cc[:, None] % 8, kk[None, :] % 8], 0.0)  # [c,k]
        qf = Q64[j8[:, None], cc[None, :] % 8]           # [jj, c]
        W2[:, q, :] = (np.einsum('ji,ck,jc->jcik', a, b, qf) / (SF * SF)).reshape(128, 256)

    # quant bias per partition p=(ii,k), per chunk parity par=p_%2:
    # bias = MAGIC + c, c = -128*Sx[i8]*Sx[k8] / (sf^2*Q50[i8,k8]); Sx[i]=0
    # for i>=1 so only DC partitions (par=0, ii=0, k%8==0) get c = -64.
    Sx = X64.sum(axis=1)
    qbias = np.full((128, 2), MAGIC, dtype=np.float64)
    for par in range(2):
        for ii in range(4):
            i8 = ii + 4 * par
            for k in range(32):
                p = ii * 32 + k
                c = -128.0 * Sx[i8] * Sx[k % 8] / (SF * SF) / Q64[i8, k % 8]
                qbias[p, par] = MAGIC + np.float64(np.float32(c))
    return (np.ascontiguousarray(W1.astype(np.float32).reshape(128, 16 * 128)),
            np.ascontiguousarray(W2.astype(np.float32).reshape(128, 8 * 256)),
            np.ascontiguousarray(qbias.astype(np.float32)))


def _build_nc(reps=1):
    import concourse.bacc as bacc
    import concourse.mybir as mybir
    from concourse import tile
    from concourse import bass
    from concourse.masks import make_identity

    f32 = mybir.dt.float32
    f32r = mybir.dt.float32r
    bf16 = mybir.dt.bfloat16
    i16 = mybir.dt.int16
    Alu = mybir.AluOpType
    Copy = mybir.ActivationFunctionType.Copy

    nc = bacc.Bacc("TRN2", target_bir_lowering=False, debug=False,
                   num_devices=N_CORES)
    x = nc.dram_tensor("x", [PER_CORE, 1024], f32r, kind="ExternalInput")
    w1 = nc.dram_tensor("w1", [128, 2048], f32r, kind="ExternalInput")
    w2 = nc.dram_tensor("w2", [128, 2048], bf16, kind="ExternalInput")
    qv = nc.dram_tensor("qv", [128, 2], f32, kind="ExternalInput")
    y = nc.dram_tensor("y", [PER_CORE, 1024], i16, kind="ExternalOutput")

    with tile.TileContext(nc) as tc, ExitStack() as ctx:
        consts = ctx.enter_context(tc.tile_pool(name="consts", bufs=1))
        iop = ctx.enter_context(tc.tile_pool(name="io", bufs=6))
        vp = ctx.enter_context(tc.tile_pool(name="v", bufs=2))
        up = ctx.enter_context(tc.tile_pool(name="u", bufs=3))
        ptp = ctx.enter_context(tc.tile_pool(name="ptp", bufs=2, space=bass.MemorySpace.PSUM))
        pt22 = ctx.enter_context(tc.tile_pool(name="pt22", bufs=2, space=bass.MemorySpace.PSUM))
        pph = ctx.enter_context(tc.tile_pool(name="pph", bufs=2, space=bass.MemorySpace.PSUM))

        w1_sb = consts.tile([128, 2048], f32r)
        w2_sb = consts.tile([128, 2048], bf16)
        qv_sb = consts.tile([128, 2], f32)
        ident0 = consts.tile([128, 128], f32)
        identr = consts.tile([128, 128], f32r)
        nc.sync.dma_start(w1_sb[:], w1[:])
        nc.sync.dma_start(w2_sb[:], w2[:])
        nc.sync.dma_start(qv_sb[:], qv[:])
        make_identity(nc, ident0[:])
        nc.scalar.copy(identr[:], ident0[:])

        def group_body(g):
            base = g * GROUP * TI
            xins = []
            for sub in range(GROUP):
                xin = iop.tile([TI, 1024], f32r, tag="xin")
                xins.append(xin)
                nc.sync.dma_start(xin[:], x[base + sub * TI:base + (sub + 1) * TI, :])

            # transpose all subs into V [128, 8, 500] f32r
            V = vp.tile([128, 8, GROUP * TI], f32r, tag="V")
            for sub in range(GROUP):
                tp = ptp.tile([128, 8, 128], f32, tag="tp")
                for q in range(8):
                    nc.tensor.transpose(
                        tp[:, q, :].bitcast(f32r),
                        xins[sub][:, q * 128:(q + 1) * 128],
                        identr[0:TI, 0:128])
                nc.scalar.copy(V[:, :, sub * TI:(sub + 1) * TI], tp[:, :, 0:TI])

            # forward + quantize: t22' = (X B X^T)/(sf^2 Q50), q = round(t22'+c)
            qt = vp.tile([128, 8, GROUP * TI], bf16, tag="qt")
            for p_ in range(8):
                jb = p_ // 2
                par = p_ % 2
                t22 = pt22.tile([128, GROUP * TI], f32, tag="t22")
                for qi in range(2):
                    q = 2 * jb + qi
                    m = p_ * 2 + qi
                    nc.tensor.matmul(
                        t22[:],
                        w1_sb[:, m * 128:(m + 1) * 128],
                        V[:, q, :],
                        start=(qi == 0), stop=(qi == 1))
                nc.vector.tensor_scalar(
                    qt[:, p_, :], t22[:],
                    qv_sb[:, par:par + 1], MAGIC, Alu.add, Alu.subtract)

            # inverse per sub, PSUM half-tiles [125, 512]
            for sub in range(GROUP):
                yi = iop.tile([TI, 1024], i16, tag="yi")
                for jbh in range(2):
                    ph = pph.tile([TI, 512], f32, tag="ph")
                    for j2 in range(2):
                        jb = 2 * jbh + j2
                        for qi in range(2):
                            q = 2 * jb + qi
                            nc.tensor.matmul(
                                ph[:, j2 * 256:(j2 + 1) * 256],
                                qt[:, q, sub * TI:(sub + 1) * TI],
                                w2_sb[:, q * 256:(q + 1) * 256],
                                start=(qi == 0), stop=(qi == 1))
                    if sub < 2:
                        # ACT (+MAGIC) then Pool (-(MAGIC-128) -> i16)
                        u2 = up.tile([TI, 512], f32, tag="u2")
                        nc.scalar.activation(u2[:], ph[:], Copy, bias=MAGIC)
                        nc.gpsimd.tensor_scalar_sub(
                            yi[:, jbh * 512:(jbh + 1) * 512], u2[:],
                            MAGIC - 128.0)
                    else:
                        nc.vector.tensor_scalar(
                            yi[:, jbh * 512:(jbh + 1) * 512], ph[:],
                            MAGIC, MAGIC - 128.0, Alu.add, Alu.subtract)
                nc.scalar.dma_start(y[base + sub * TI:base + (sub + 1) * TI, :],
                                    yi[:])

        def body():
            for g in range(NG):
                group_body(g)

        if reps == 1:
            body()
        else:
            with tc.For_i(0, reps, 1):
                body()

    nc.compile()
    return nc


_NC_CACHE = None
PROFILE = False
LAST_RESULT = None


def kernel(**inputs) -> np.ndarray:
    global _NC_CACHE, LAST_RESULT
    from concourse.bass_utils import run_bass_kernel_spmd
    import ml_dtypes

    x = np.ascontiguousarray(np.asarray(inputs["input"], dtype=np.float32))
    S = x.shape[0]
    assert S == SIZE, f"expected {SIZE} images, got {S}"
    xf = x.reshape(N_CORES, PER_CORE, 1024)

    w = inputs.get("weight")
    wf = inputs.get("weight_factor")
    wfv = float(np.asarray(wf).reshape(-1)[0]) if wf is not None else 65000.0
    if w is not None:
        w = np.asarray(w, dtype=np.float32)
        assert w.shape[0] == 1, "kernel supports n_mult=1"
        w = w[0]
    W1, W2, qbias = _build_constants(w, wfv)
    W2 = np.ascontiguousarray(W2.astype(ml_dtypes.bfloat16))

    if _NC_CACHE is None:
        _NC_CACHE = _build_nc()
    nc = _NC_CACHE

    in_maps = [
        {"x": np.ascontiguousarray(xf[c]), "w1": W1, "w2": W2, "qv": qbias}
        for c in range(N_CORES)
    ]
    res = run_bass_kernel_spmd(nc, in_maps, core_ids=list(range(N_CORES)),
                               trace=PROFILE)
    LAST_RESULT = res
    out = np.stack([res.results[c]["y"] for c in range(N_CORES)], axis=0)
    return out.reshape(1, 1, SIZE, 32, 32).astype(np.float32)


if __name__ == "__main__":
    rng = np.random.default_rng(0)
    x = (rng.random((SIZE, 1, 32, 32)) * 255).astype(np.float32)
    y = kernel(input=x)
    print("kernel ran, out shape", y.shape, y.dtype)


# revision 8
# speedup vs baseline: 1.5751x; 1.5751x over previous
"""Trainium2 Bass kernel for nn_DCT_Forward_Model (JPEG-style DCT quantize/dequantize).

Math: the reference output equals the approx_dct forward path:
  B = img - 128 (per 8x8 block), t22 = (X @ B @ X^T)/sf^2 with X = f32(D*65000),
  q = round(t22/Q50 + 1e-6), deq = Q50*q, t2 = (X^T @ deq @ X)/sf^2, out = round(t2)+128.
(The grad path g cancels: out = g + stopgrad(a - g) == a up to fp noise.)

v3 design (per NeuronCore, pure data parallel, 2500 images/core):
  - host-side pixel permutation groups each image's 1024 pixels as
    (jb, cbp, r, c16): 128 consecutive cols = TWO complete 8x8 blocks
    -> the 2D DCT becomes ONE dense-interleaved 128x128 matmul per pair
    (2x PE utilization vs chunked Kronecker form)
  - DMA: 4-row image interleave => 16KB load descriptors / 8KB store
    descriptors; per-ring HW bandwidth is descriptor-bound (~105 GB/s at
    4KB), loads alternate sync/scalar HWDGE rings, stores on gpsimd SWDGE
  - PE: transposes (f32r, ~96ns), forward = 8 MM/group with ONE shared
    [128,128] f32r weight (1/Q50 folded in), inverse = 32 single MMs bf16
    with ONE shared [128,128] rhs
  - quantize: ONE DVE tensor_scalar (add MAGIC+c, sub MAGIC) -> bf16;
    c = -64 on the two block-DC partitions folds the -128 shift exactly
  - output: ONE op (add 128 -> int16 cast) per PSUM half, split ACT/DVE;
    the write AP un-permutes pixels back to natural order
"""

import os
import sys
import numpy as np
from contextlib import ExitStack

if "/opt/trn_rl_repo" not in sys.path and os.path.isdir("/opt/trn_rl_repo"):
    sys.path.insert(0, "/opt/trn_rl_repo")

N_CORES = 8
SIZE = 20000
PER_CORE = SIZE // N_CORES  # 2500
TI = 125                    # images per s-slice
GROUP = 4                   # s-slices per group (4-row DMA interleave)
NG = PER_CORE // (GROUP * TI)  # 5 groups
MAGIC = 12582912.0          # 1.5 * 2^23: fp32 add snaps to integer (RNE)
STAGES = ("tr", "fwd", "inv")  # ablation control (bench only)

_Q50 = np.array(
    [[16, 11, 10, 16, 24, 40, 51, 61], [12, 12, 14, 19, 26, 58, 60, 55],
     [14, 13, 16, 24, 40, 57, 69, 56], [14, 17, 22, 29, 51, 87, 80, 62],
     [18, 22, 37, 56, 68, 109, 103, 77], [24, 35, 55, 64, 81, 104, 113, 92],
     [49, 64, 78, 87, 103, 121, 120, 101], [72, 92, 95, 98, 112, 100, 103, 99]],
    dtype=np.float32)


def _dct_mat8():
    k = np.arange(8)[:, None]
    n = np.arange(8)[None, :]
    D = np.cos(np.pi * k * (2 * n + 1) / 16.0)
    D[0] *= np.sqrt(1.0 / 8.0)
    D[1:] *= np.sqrt(2.0 / 8.0)
    return D.astype(np.float32)


def _pixel_perm():
    """perm[newcol] = oldcol: newcol = jb*256 + cbp*128 + r*16 + c16,
    oldcol = (8*jb + r)*32 + 16*cbp + c16."""
    jb, cbp, r, c = np.meshgrid(np.arange(4), np.arange(2), np.arange(8),
                                np.arange(16), indexing="ij")
    return ((8 * jb + r) * 32 + 16 * cbp + c).reshape(-1)


def _build_constants(weight=None, wf=65000.0):
    SF = np.float64(wf)
    if weight is None:
        Xbase = _dct_mat8()
    else:
        Xbase = np.asarray(weight, dtype=np.float32).reshape(8, 8)
    X = (Xbase * np.float32(wf)).astype(np.float32)
    X64 = X.astype(np.float64)
    Q64 = _Q50.astype(np.float64)

    # Pair-packed index: partition k = r*16 + c'' (r in 8, c'' in 16; block
    # b = c''//8, col-in-block = c''%8); output m = i*16 + j'' (same form).
    # W1p[k=(r,c''), m=(i,j'')] = X[i,r]*X[j''%8,c''%8]*[same block]
    #                             / (sf^2 * Q50[i,j''%8])
    W1p = np.zeros((8, 16, 8, 16), dtype=np.float64)
    W2p = np.zeros((8, 16, 8, 16), dtype=np.float64)
    for i in range(8):
        for j2 in range(16):
            j8 = j2 % 8
            for rr in range(8):
                for c2 in range(16):
                    if c2 // 8 != j2 // 8:
                        continue
                    W1p[rr, c2, i, j2] = (X64[i, rr] * X64[j8, c2 % 8]
                                          / (SF * SF) / Q64[i, j8])
                    W2p[i, j2, rr, c2] = (X64[i, rr] * X64[j8, c2 % 8]
                                          * Q64[i, j8] / (SF * SF))
    W1p = W1p.reshape(128, 128)
    W2p = W2p.reshape(128, 128)

    # quant bias per output partition m=(i,j''): MAGIC + c,
    # c = -128*Sx[i]*Sx[j8]/(sf^2*Q50[i,j8]) (== -64 iff i==0 and j8==0
    # for the stock DCT; computed generally for arbitrary weights).
    Sx = X64.sum(axis=1)
    qbias = np.full((128, 1), MAGIC, dtype=np.float64)
    for i in range(8):
        for j2 in range(16):
            c = -128.0 * Sx[i] * Sx[j2 % 8] / (SF * SF) / Q64[i, j2 % 8]
            qbias[i * 16 + j2, 0] = MAGIC + np.float64(np.float32(c))
    return (np.ascontiguousarray(W1p.astype(np.float32)),
            np.ascontiguousarray(W2p.astype(np.float32)),
            np.ascontiguousarray(qbias.astype(np.float32)))


def _build_nc(reps=1):
    import concourse.bacc as bacc
    import concourse.mybir as mybir
    from concourse import tile
    from concourse import bass
    from concourse.masks import make_identity

    f32 = mybir.dt.float32
    f32r = mybir.dt.float32r
    bf16 = mybir.dt.bfloat16
    i16 = mybir.dt.int16
    Alu = mybir.AluOpType
    Copy = mybir.ActivationFunctionType.Copy

    nc = bacc.Bacc("TRN2", target_bir_lowering=False, debug=False,
                   num_devices=N_CORES)
    x = nc.dram_tensor("x", [PER_CORE, 1024], f32r, kind="ExternalInput")
    w1 = nc.dram_tensor("w1", [128, 128], f32r, kind="ExternalInput")
    w2 = nc.dram_tensor("w2", [128, 128], bf16, kind="ExternalInput")
    qv = nc.dram_tensor("qv", [128, 1], f32, kind="ExternalInput")
    y = nc.dram_tensor("y", [PER_CORE, 1024], i16, kind="ExternalOutput")

    GI = GROUP * TI  # 500 images per group

    with tile.TileContext(nc) as tc, ExitStack() as ctx:
        consts = ctx.enter_context(tc.tile_pool(name="consts", bufs=1))
        iop = ctx.enter_context(tc.tile_pool(name="io", bufs=3))
        vp = ctx.enter_context(tc.tile_pool(name="v", bufs=2))
        ptp = ctx.enter_context(tc.tile_pool(name="ptp", bufs=2, space=bass.MemorySpace.PSUM))
        pt22 = ctx.enter_context(tc.tile_pool(name="pt22", bufs=2, space=bass.MemorySpace.PSUM))
        pph = ctx.enter_context(tc.tile_pool(name="pph", bufs=2, space=bass.MemorySpace.PSUM))

        w1_sb = consts.tile([128, 128], f32r)
        w2_sb = consts.tile([128, 128], bf16)
        qv_sb = consts.tile([128, 1], f32)
        ident0 = consts.tile([128, 128], f32)
        identr = consts.tile([128, 128], f32r)
        nc.sync.dma_start(w1_sb[:], w1[:])
        nc.sync.dma_start(w2_sb[:], w2[:])
        nc.sync.dma_start(qv_sb[:], qv[:])
        make_identity(nc, ident0[:])
        nc.scalar.copy(identr[:], ident0[:])

        def group_body(g):
            base = g * GI
            # one 2MB load; image 4p+s lives at partition p, slot s
            xin = iop.tile([TI, GROUP, 1024], f32r, tag="xin")
            eng_l = nc.sync if g % 2 == 0 else nc.scalar
            eng_l.dma_start(
                xin[:],
                x[base:base + GI, :].rearrange("(p s) f -> p s f", s=GROUP))

            yi = iop.tile([TI, GROUP, 1024], i16, tag="yi")

            def fake_out():
                for s in range(GROUP):
                    nc.vector.tensor_copy(yi[:, s, :],
                                          xin[:, s, 0:512].bitcast(i16))
                nc.gpsimd.dma_start(
                    y[base:base + GI, :].rearrange("(p s) f -> p s f", s=GROUP),
                    yi[:])

            if "tr" not in STAGES:
                fake_out()
                return

            # transposes: per s-slice, 8 block-pair tensors -> V [128,8,500]
            V = vp.tile([128, 8, GI], f32r, tag="V")
            for s in range(GROUP):
                tp = ptp.tile([128, 8, 128], f32, tag="tp")
                for t in range(8):
                    nc.tensor.transpose(
                        tp[:, t, :].bitcast(f32r),
                        xin[:, s, t * 128:(t + 1) * 128],
                        identr[0:TI, 0:128])
                if s < 2:
                    nc.scalar.copy(V[:, :, s * TI:(s + 1) * TI], tp[:, :, 0:TI])
                else:
                    nc.vector.tensor_copy(V[:, :, s * TI:(s + 1) * TI],
                                          tp[:, :, 0:TI])

            if "fwd" not in STAGES:
                fake_out()
                return

            # forward: ONE shared f32r weight; 8 MMs of N=500; quant -> bf16
            qt = vp.tile([128, 8, GI], bf16, tag="qt")
            for t in range(8):
                t22 = pt22.tile([128, GI], f32, tag="t22")
                nc.tensor.matmul(t22[:], w1_sb[:], V[:, t, :],
                                 start=True, stop=True)
                nc.vector.tensor_scalar(
                    qt[:, t, :], t22[:], qv_sb[:, 0:1], MAGIC,
                    Alu.add, Alu.subtract)

            if "inv" not in STAGES:
                fake_out()
                return

            # inverse: per s-slice, two PSUM halves of 4 block-pairs each
            for s in range(GROUP):
                for h in range(2):
                    ph = pph.tile([TI, 4, 128], f32, tag="ph")
                    for u in range(4):
                        t = h * 4 + u  # block-pair (jb = t//2, cbp = t%2)
                        nc.tensor.matmul(
                            ph[:, u, :],
                            qt[:, t, s * TI:(s + 1) * TI],
                            w2_sb[:],
                            start=True, stop=True)
                    # un-permute pixels: src (jb,cbp,r,c16) -> natural col
                    # (8jb+r)*32 + 16cbp + c16; split per jb (3 free dims max)
                    for j_ in range(2):
                        src = ph[:, 2 * j_:2 * j_ + 2, :].rearrange(
                            "p cbp (r c) -> p cbp r c", r=8)
                        dst = yi[:, s, h * 512 + j_ * 256:
                                 h * 512 + (j_ + 1) * 256].rearrange(
                            "p (r cbp c) -> p cbp r c", r=8, cbp=2)
                        if s % 2 == 0:
                            nc.scalar.activation(dst, src, Copy, bias=128.0)
                        else:
                            nc.vector.tensor_scalar_add(dst, src, 128.0)
            nc.gpsimd.dma_start(
                y[base:base + GI, :].rearrange("(p s) f -> p s f", s=GROUP),
                yi[:])

        def body():
            for g in range(NG):
                group_body(g)

        if reps == 1:
            body()
        else:
            with tc.For_i(0, reps, 1):
                body()

    nc.compile()
    return nc


_NC_CACHE = None
PROFILE = False
LAST_RESULT = None


def kernel(**inputs) -> np.ndarray:
    global _NC_CACHE, LAST_RESULT
    from concourse.bass_utils import run_bass_kernel_spmd
    import ml_dtypes

    x = np.ascontiguousarray(np.asarray(inputs["input"], dtype=np.float32))
    S = x.shape[0]
    assert S == SIZE, f"expected {SIZE} images, got {S}"
    xf = x.reshape(N_CORES, PER_CORE, 1024)[:, :, _pixel_perm()]

    w = inputs.get("weight")
    wf = inputs.get("weight_factor")
    wfv = float(np.asarray(wf).reshape(-1)[0]) if wf is not None else 65000.0
    if w is not None:
        w = np.asarray(w, dtype=np.float32)
        assert w.shape[0] == 1, "kernel supports n_mult=1"
        w = w[0]
    W1p, W2p, qbias = _build_constants(w, wfv)
    W2p = np.ascontiguousarray(W2p.astype(ml_dtypes.bfloat16))

    if _NC_CACHE is None:
        _NC_CACHE = _build_nc()
    nc = _NC_CACHE

    in_maps = [
        {"x": np.ascontiguousarray(xf[c]), "w1": W1p, "w2": W2p, "qv": qbias}
        for c in range(N_CORES)
    ]
    res = run_bass_kernel_spmd(nc, in_maps, core_ids=list(range(N_CORES)),
                               trace=PROFILE)
    LAST_RESULT = res
    out = np.stack([res.results[c]["y"] for c in range(N_CORES)], axis=0)
    return out.reshape(1, 1, SIZE, 32, 32).astype(np.float32)


if __name__ == "__main__":
    rng = np.random.default_rng(0)
    x = (rng.random((SIZE, 1, 32, 32)) * 255).astype(np.float32)
    y = kernel(input=x)
    print("kernel ran, out shape", y.shape, y.dtype)
